# revision 26
# baseline (speedup 1.0000x reference)
"""Conditional BatchNorm1d (training-mode, per-class stats) on 8 Trainium2
NeuronCores.

Problem: x [512, 128, 1024] f32, labels [512] i32 in [0,8), weight/bias
[8, 128] f32.  Per-class biased mean/var over the class's (batch, length)
elements per feature, then per-class affine:
    y = x * (rsqrt(var+eps)*w)[lbl] + (b - mean*rsqrt(var+eps)*w)[lbl]

Sharding: data-parallel over batch B across the 8 cores (64 batches each).

v6 (from v4 @154us): phase 1 was engine-bound (DVE ran 13us past the
last DMA).  (v5 tried an fp16 SBUF scratch to get the ACT lane to 2x
rate -- the perf mode does not engage for activation+accum, so the
22/42 ACT/DVE split stays.)
  * Merges run per 4-group block (4x5 wide DVE ops instead of 16x5
    tiny ones), cutting ~7us of DVE bubble overhead.
  * Stats columns live in engine-major permuted order (DVE batches in
    cols 0:32, ACT in 32:64) so block merges and the transposes stay
    contiguous; the host permutes the class-mask rows to match.
  * Consts issue from the idle PE sequencer (ACT sequencer fed the
    in-order const DMAs before).
  * Tiny warmup AllGather at t~1us eats the CC-stack first-op cost
    (~64us!) off the critical path; the real stats collective is an
    AllGather (12us flight vs 30us AllReduce) + local selection matmul.
  * Pass 2 all-DVE; loads/stores alternate sync/gpsimd DMA queues.

Layout: feature-major shard [F=128, B_LOC=64, L=1024] fp16; GRP=4
batches per DMA keeps 8 KiB of DRAM-contiguous data per partition.
"""

import sys

if "/opt/trn_rl_repo" not in sys.path:
    sys.path.insert(0, "/opt/trn_rl_repo")

import numpy as np

import concourse.bacc as bacc
import concourse.tile as tile
from concourse import mybir
from concourse import bass_utils

B, F, L = 512, 128, 1024
K = 8
N_CORES = 8
B_LOC = B // N_CORES  # 64
EPS = 1e-5
GRP = 4               # batches per DMA group (8 KiB/partition fp16)
N_GRP = B_LOC // GRP  # 16
# Groups whose SECOND batch also goes to the ACT lane (ACT covers
# batches [4g, 4g+n_act(g)); DVE bn_stats covers the rest).  22 ACT
# batches / 42 DVE batches drains both engines with the DMA stream.
ACT2 = frozenset({1, 3, 5, 9, 11, 13})
N_ACT_G = [2 if g in ACT2 else 1 for g in range(N_GRP)]
N_DVE_G = [GRP - n for n in N_ACT_G]
DVE_OFF = [0]
for g in range(N_GRP):
    DVE_OFF.append(DVE_OFF[-1] + N_DVE_G[g])
NB_DVE = DVE_OFF[-1]      # 42 DVE-lane batches
ACT_OFF = [NB_DVE]
for g in range(N_GRP):
    ACT_OFF.append(ACT_OFF[-1] + N_ACT_G[g])
MERGE_BLK = 4             # groups per merge block

F32 = mybir.dt.float32
F16 = mybir.dt.float16
AFT = mybir.ActivationFunctionType
AX = mybir.AxisListType
ALU = mybir.AluOpType

_built = None


def _stats_col(g, i):
    """Column in the permuted stats array for batch (group g, lane i)."""
    if i < N_ACT_G[g]:
        return ACT_OFF[g] + i                # ACT region
    return DVE_OFF[g] + (i - N_ACT_G[g])     # DVE region


def _build():
    nc = bacc.Bacc("TRN2", target_bir_lowering=False, debug=False,
                   num_devices=N_CORES)

    x = nc.dram_tensor("x", [F, B_LOC, L], F16, kind="ExternalInput")
    # One-hot label mask, transposed: maskT[k, b] = 1 iff labels[b] == k
    maskT = nc.dram_tensor("maskT", [K, B_LOC], F32, kind="ExternalInput")
    # Per-class stats mask in PERMUTED row order, rcp-scaled:
    # mask64[p, k] = 256/cnt_k iff labels[batch_at_col_p] == k.
    mask64 = nc.dram_tensor("mask64", [B_LOC, K], F32, kind="ExternalInput")
    # Cross-core sum-of-shards selector: selT[8r+k, k'] = (k == k').
    selT = nc.dram_tensor("selT", [N_CORES * K, K], F32,
                          kind="ExternalInput")
    ident = nc.dram_tensor("ident", [128, 128], F32, kind="ExternalInput")
    epsv = nc.dram_tensor("epsv", [K, 1], F32, kind="ExternalInput")
    weight = nc.dram_tensor("weight", [K, F], F32, kind="ExternalInput")
    bias = nc.dram_tensor("bias", [K, F], F32, kind="ExternalInput")
    y = nc.dram_tensor("y", [F, B_LOC, L], F16, kind="ExternalOutput")

    groups = [list(range(N_CORES))]

    with tile.TileContext(nc) as tc:
        with (
            tc.tile_pool(name="const", bufs=1) as constp,
            tc.tile_pool(name="xres", bufs=N_GRP) as xres,
            tc.tile_pool(name="stats", bufs=1) as statsp,
            tc.tile_pool(name="pscr", bufs=2, space="PSUM") as pscr,
            tc.tile_pool(name="psmall", bufs=3, space="PSUM") as psmall,
            tc.tile_pool(name="dram", bufs=1, space="DRAM") as dram,
            tc.tile_pool(name="yout", bufs=4) as yout,
        ):
            # ---- warmup collective: absorbs the CC-stack first-op cost
            # (~64us) + launch skew while the x stream loads.
            warm_in = dram.tile([1, 8], F32)
            warm_out = dram.tile([K, 8], F32, addr_space="Shared")
            wz = statsp.tile([1, 8], F32)
            nc.gpsimd.memset(wz[:], 0.0)
            nc.gpsimd.dma_start(warm_in[:], wz[:])
            nc.gpsimd.collective_compute(
                "AllGather", ALU.bypass, replica_groups=groups,
                ins=[warm_in.opt()], outs=[warm_out.opt()])

            # const loads issue from the ACT sequencer so the x loads
            # lead the in-order Sync stream (PE cannot initiate DMAs).
            cpack1 = constp.tile([128, 128], F32)
            identt = cpack1[:, 0:128]
            nc.scalar.dma_start(identt, ident[:])
            cpack2 = constp.tile([B_LOC, 16], F32)
            mask64t = cpack2[:, 0:K]
            selTt = cpack2[:, K:2 * K]
            nc.scalar.dma_start(mask64t, mask64[:])
            nc.scalar.dma_start(selTt, selT[:])
            cpack4 = constp.tile([K, 321], F32)
            maskTt = cpack4[:, 0:B_LOC]
            wt = cpack4[:, 64:192]
            bt = cpack4[:, 192:320]
            epst = cpack4[:, 320:321]
            nc.scalar.dma_start(maskTt, maskT[:])
            nc.scalar.dma_start(wt, weight[:])
            nc.scalar.dma_start(bt, bias[:])
            nc.scalar.dma_start(epst, epsv[:])

            # ---- stats tiles (engine-major permuted column order) ----
            # cols 0:42  = DVE-lane batches (in group order)
            # cols 42:64 = ACT-lane batches (in group order)
            spackD = statsp.tile([128, 128 + NB_DVE], F32)
            Scol = spackD[:, 0:B_LOC]
            Qcol = spackD[:, B_LOC:128]
            CVcol = spackD[:, 128:128 + NB_DVE]
            # bn_stats raw out: [f, dve_batch, chunk, parity, (cnt,mean,cv)]
            BS = statsp.tile([128, NB_DVE, 2, 2, 3], F16)
            psq = statsp.tile([128, NB_DVE, 2, 2, 1], F32)
            # batch-major transposed stats: cols 0:128 = S^T, 128:256 = Q^T
            sqt = statsp.tile([B_LOC, 256], F32)
            gpart = statsp.tile([K, 256], F32)
            Gall = statsp.tile([N_CORES * K, 256], F32)
            gred = statsp.tile([K, 256], F32)
            postt = statsp.tile([K, 640], F32)
            selc = statsp.tile([128, 128], F32)

            # per-batch merge of bn_stats sub-chunk stats over a block of
            # DVE columns [lo, hi): sum(x)/256 = sum(means);
            # sum(x^2)/256 = sum(cv)/256 + sum(means^2).
            def merge(lo, hi):
                means = BS[:, lo:hi, :, :, 1:2]
                cvs = BS[:, lo:hi, :, :, 2:3]
                nc.vector.tensor_reduce(Scol[:, lo:hi], means, axis=AX.XYZ,
                                        op=ALU.add)
                nc.vector.tensor_mul(psq[:, lo:hi], means, means)
                nc.vector.tensor_reduce(Qcol[:, lo:hi], psq[:, lo:hi],
                                        axis=AX.XYZ, op=ALU.add)
                nc.vector.tensor_reduce(CVcol[:, lo:hi], cvs, axis=AX.XYZ,
                                        op=ALU.add)
                nc.vector.scalar_tensor_tensor(
                    Qcol[:, lo:hi], CVcol[:, lo:hi], 1.0 / 256.0,
                    Qcol[:, lo:hi], ALU.mult, ALU.add)

            # three load queues: sync and gpsimd issue up-front; ACT's
            # issues are interleaved two groups ahead of its stats stream
            # so they don't delay the activations.
            res_tiles = {}
            for g in range(N_GRP):
                xtile = xres.tile([F, GRP * L], F16, tag="xs", name=f"xs{g}")
                res_tiles[g] = xtile

            def load_issue(g):
                ldq = (nc.sync, nc.gpsimd, nc.scalar)[g % 3]
                ldq.dma_start(res_tiles[g][:],
                              x[:, g * GRP:(g + 1) * GRP, :])

            for g in range(N_GRP):
                if g % 3 != 2:
                    load_issue(g)
            load_issue(2)

            for g in range(N_GRP):
                xt = res_tiles[g]
                if g % 3 == 2 and g + 3 < N_GRP:
                    load_issue(g + 3)  # next ACT-queue load, 3 groups ahead
                for i in range(N_ACT_G[g]):
                    col = _stats_col(g, i)
                    xs = xt[:, i * L:(i + 1) * L]
                    # scale folds the 1/256 unit: (x/16)^2 and x/256.
                    scr = pscr.tile([128, L], F32, tag="ascr")
                    nc.scalar.activation(scr[:], xs, AFT.Square,
                                         scale=0.0625,
                                         accum_out=Qcol[:, col:col + 1])
                    scr2 = pscr.tile([128, L], F32, tag="ascr")
                    nc.scalar.activation(scr2[:], xs, AFT.Identity,
                                         scale=1.0 / 256.0,
                                         accum_out=Scol[:, col:col + 1])
                # bn_stats is capped at 512 free elems per op: 2 ops per
                # DVE batch
                for j in range(N_DVE_G[g]):
                    db = DVE_OFF[g] + j
                    i = N_ACT_G[g] + j
                    for c in range(2):
                        nc.vector.bn_stats(
                            BS[:, db:db + 1, c:c + 1, :, :],
                            xt[:, (2 * i + c) * 512:(2 * i + c + 1) * 512])
                if g % MERGE_BLK == MERGE_BLK - 1:
                    blk = g // MERGE_BLK
                    merge(DVE_OFF[blk * MERGE_BLK],
                          DVE_OFF[(blk + 1) * MERGE_BLK])

            # Pre-load the Sqrt ACT table while ACT idles (the implicit
            # table swap would otherwise land on the post-AllGather
            # critical path).
            dumt = statsp.tile([K, 1], F32)
            nc.scalar.activation(dumt[:], epst, AFT.Sqrt)

            # ---- local per-class reduction: transpose + masked matmul ----
            sq_ps = psmall.tile([B_LOC, 256], F32, tag="ps")
            nc.tensor.transpose(sq_ps[:, 0:128], Scol, identt)
            nc.tensor.transpose(sq_ps[:, 128:256], Qcol, identt)
            nc.vector.tensor_copy(sqt[:], sq_ps[:])
            # gpart[k, 0:128] = partial mean, [k, 128:256] = partial E[x^2]
            # (mask64 carries the global 256/cnt factor, permuted rows).
            gp_ps = psmall.tile([K, 256], F32, tag="ps")
            nc.tensor.matmul(gp_ps[:], mask64t, sqt[:], start=True,
                             stop=True)
            nc.vector.tensor_copy(gpart[:], gp_ps[:])

            # ---- all-gather the [8, 256] partials across the 8 cores ----
            cc_in = dram.tile([K, 256], F32)
            cc_out = dram.tile([N_CORES * K, 256], F32, addr_space="Shared")
            # upload via GpSimd: a wait on the in-order Sync stream would
            # stall the stores queued there.
            nc.gpsimd.dma_start(cc_in[:], gpart[:])
            nc.gpsimd.collective_compute(
                "AllGather", ALU.bypass, replica_groups=groups,
                ins=[cc_in.opt()], outs=[cc_out.opt()])
            # Download issues from the ACT sequencer: it must wait for the
            # AllGather, and ACT is idle here anyway.
            nc.scalar.dma_start(Gall[:], cc_out[:])

            # ---- global stats + scale/shift per (class, feature) ----
            gr_ps = psmall.tile([K, 256], F32, tag="ps")
            nc.tensor.matmul(gr_ps[:], selTt, Gall[:], start=True,
                             stop=True)
            nc.vector.tensor_copy(gred[:], gr_ps[:])
            Gs = gred[:, 0:128]
            Gq = gred[:, 128:256]
            t0 = postt[:, 0:128]
            var = postt[:, 128:256]
            std = postt[:, 256:384]
            inv = postt[:, 384:512]
            scal = postt[:, 512:640]
            shft = t0  # reuse: t0 is dead once var is computed
            nc.vector.tensor_mul(t0, Gs, Gs)
            nc.vector.tensor_sub(var, Gq, t0)
            nc.scalar.activation(std, var, AFT.Sqrt, bias=epst)
            nc.vector.reciprocal(inv, std)
            nc.vector.tensor_mul(scal, inv, wt)

            # ---- select: [f, 0:64] = scale col, [f, 64:128] = shift col
            # (the scale-select matmul overlaps the shift math on DVE)
            sel_ps = psmall.tile([128, 2 * B_LOC], F32, tag="ps")
            nc.tensor.matmul(sel_ps[:, 0:B_LOC], scal, maskTt, start=True,
                             stop=True)
            nc.vector.tensor_mul(shft, Gs, scal)
            nc.vector.tensor_sub(shft, bt, shft)
            nc.tensor.matmul(sel_ps[:, B_LOC:2 * B_LOC], shft, maskTt,
                             start=True, stop=True)
            nc.vector.tensor_copy(selc[:], sel_ps[:])

            # ---- pass 2: y[:, b] = x[:, b] * ssel[:, b] + tsel[:, b] ----
            # All-DVE: ~0.5us/batch fp16 keeps compute well ahead of the
            # DMA drain; stores alternate between the gpsimd and sync
            # queues.
            for g in range(N_GRP):
                xt = res_tiles[g]
                yt = yout.tile([F, GRP * L], F16)
                for i in range(GRP):
                    b = g * GRP + i
                    nc.vector.tensor_scalar(yt[:, i * L:(i + 1) * L],
                                            xt[:, i * L:(i + 1) * L],
                                            selc[:, b:b + 1],
                                            selc[:, B_LOC + b:B_LOC + b + 1],
                                            ALU.mult, ALU.add)
                stq = (nc.gpsimd, nc.sync, nc.scalar)[g % 3]
                stq.dma_start(y[:, g * GRP:(g + 1) * GRP, :], yt[:])

    nc.finalize()
    return nc


def _get_nc():
    global _built
    if _built is None:
        _built = _build()
    return _built


def _host_inputs(x, labels, weight, bias):
    labels = np.asarray(labels).astype(np.int64)
    counts = np.bincount(labels, minlength=K).astype(np.float64) * L
    rcp = (256.0 / np.maximum(counts, 1.0)).astype(np.float32)  # [K]
    ident = np.eye(128, dtype=np.float32)
    selT = np.tile(np.eye(K, dtype=np.float32), (N_CORES, 1))  # [64, 8]
    # permutation: stats column p holds batch perm[p]
    perm = np.empty(B_LOC, dtype=np.int64)
    for g in range(N_GRP):
        for i in range(GRP):
            perm[_stats_col(g, i)] = g * GRP + i
    xh = np.asarray(x, dtype=np.float16)

    in_maps = []
    for c in range(N_CORES):
        lab = labels[c * B_LOC:(c + 1) * B_LOC]
        onehot = np.zeros((B_LOC, K), dtype=np.float32)
        onehot[np.arange(B_LOC), lab] = 1.0
        mask64 = onehot[perm] * rcp.reshape(1, K)
        in_maps.append({
            # feature-major shard: [F, B_LOC, L] fp16
            "x": np.ascontiguousarray(
                xh[c * B_LOC:(c + 1) * B_LOC].transpose(1, 0, 2)),
            "maskT": np.ascontiguousarray(onehot.T),
            "mask64": mask64,
            "selT": selT,
            "ident": ident,
            "epsv": np.full((K, 1), EPS, dtype=np.float32),
            "weight": np.ascontiguousarray(
                np.asarray(weight, dtype=np.float32)),
            "bias": np.ascontiguousarray(
                np.asarray(bias, dtype=np.float32)),
        })
    return in_maps


def run(x, labels, weight, bias, trace=False):
    nc = _get_nc()
    in_maps = _host_inputs(x, labels, weight, bias)
    res = bass_utils.run_bass_kernel_spmd(nc, in_maps, list(range(N_CORES)),
                                          trace=trace)
    out = np.concatenate(
        [res.results[c]["y"].transpose(1, 0, 2) for c in range(N_CORES)],
        axis=0).astype(np.float32)
    return out, res


def kernel(x, labels, weight, bias):
    out, _ = run(np.asarray(x, dtype=np.float32), labels,
                 np.asarray(weight, dtype=np.float32),
                 np.asarray(bias, dtype=np.float32))
    return out


# revision 27
# speedup vs baseline: 1.0081x; 1.0081x over previous
"""Conditional BatchNorm1d (training-mode, per-class stats) on 8 Trainium2
NeuronCores.

Problem: x [512, 128, 1024] f32, labels [512] i32 in [0,8), weight/bias
[8, 128] f32.  Per-class biased mean/var over the class's (batch, length)
elements per feature, then per-class affine:
    y = x * (rsqrt(var+eps)*w)[lbl] + (b - mean*rsqrt(var+eps)*w)[lbl]

Sharding: data-parallel over batch B across the 8 cores (64 batches each).

v6 (from v4 @154us): phase 1 was engine-bound (DVE ran 13us past the
last DMA).  (v5 tried an fp16 SBUF scratch to get the ACT lane to 2x
rate -- the perf mode does not engage for activation+accum, so the
22/42 ACT/DVE split stays.)
  * Merges run per 4-group block (4x5 wide DVE ops instead of 16x5
    tiny ones), cutting ~7us of DVE bubble overhead.
  * Stats columns live in engine-major permuted order (DVE batches in
    cols 0:32, ACT in 32:64) so block merges and the transposes stay
    contiguous; the host permutes the class-mask rows to match.
  * Consts issue from the idle PE sequencer (ACT sequencer fed the
    in-order const DMAs before).
  * Tiny warmup AllGather at t~1us eats the CC-stack first-op cost
    (~64us!) off the critical path; the real stats collective is an
    AllGather (12us flight vs 30us AllReduce) + local selection matmul.
  * Pass 2 all-DVE; loads/stores alternate sync/gpsimd DMA queues.

Layout: feature-major shard [F=128, B_LOC=64, L=1024] fp16; GRP=4
batches per DMA keeps 8 KiB of DRAM-contiguous data per partition.
"""

import sys

if "/opt/trn_rl_repo" not in sys.path:
    sys.path.insert(0, "/opt/trn_rl_repo")

import numpy as np

import concourse.bacc as bacc
import concourse.tile as tile
from concourse import mybir
from concourse import bass_utils

B, F, L = 512, 128, 1024
K = 8
N_CORES = 8
B_LOC = B // N_CORES  # 64
EPS = 1e-5
GRP = 4               # batches per DMA group (8 KiB/partition fp16)
N_GRP = B_LOC // GRP  # 16
# ACT covers the LAST n_act(g) batches of each group, DVE the first
# n_dve(g); 22 ACT / 42 DVE batches drains both engines with the DMA
# stream.  The 2-ACT-batch groups sit EARLY so the last-arriving half
# group carries the least tail work (DVE: 2 bn_stats batches, ACT: 1).
N_ACT_G = [2 if g < 6 else 1 for g in range(N_GRP)]
N_DVE_G = [GRP - n for n in N_ACT_G]
DVE_OFF = [0]
for g in range(N_GRP):
    DVE_OFF.append(DVE_OFF[-1] + N_DVE_G[g])
NB_DVE = DVE_OFF[-1]      # 42 DVE-lane batches
ACT_OFF = [NB_DVE]
for g in range(N_GRP):
    ACT_OFF.append(ACT_OFF[-1] + N_ACT_G[g])
MERGE_BLK = 4             # groups per merge block

F32 = mybir.dt.float32
F16 = mybir.dt.float16
AFT = mybir.ActivationFunctionType
AX = mybir.AxisListType
ALU = mybir.AluOpType

_built = None


def _stats_col(g, i):
    """Column in the permuted stats array for batch (group g, lane i)."""
    if i >= N_DVE_G[g]:
        return ACT_OFF[g] + (i - N_DVE_G[g])  # ACT region (last batches)
    return DVE_OFF[g] + i                     # DVE region (first batches)


def _build():
    nc = bacc.Bacc("TRN2", target_bir_lowering=False, debug=False,
                   num_devices=N_CORES)

    x = nc.dram_tensor("x", [F, B_LOC, L], F16, kind="ExternalInput")
    # One-hot label mask, transposed: maskT[k, b] = 1 iff labels[b] == k
    maskT = nc.dram_tensor("maskT", [K, B_LOC], F32, kind="ExternalInput")
    # Per-class stats mask in PERMUTED row order, rcp-scaled:
    # mask64[p, k] = 256/cnt_k iff labels[batch_at_col_p] == k.
    mask64 = nc.dram_tensor("mask64", [B_LOC, K], F32, kind="ExternalInput")
    # Cross-core sum-of-shards selector: selT[8r+k, k'] = (k == k').
    selT = nc.dram_tensor("selT", [N_CORES * K, K], F32,
                          kind="ExternalInput")
    ident = nc.dram_tensor("ident", [128, 128], F32, kind="ExternalInput")
    epsv = nc.dram_tensor("epsv", [K, 1], F32, kind="ExternalInput")
    weight = nc.dram_tensor("weight", [K, F], F32, kind="ExternalInput")
    bias = nc.dram_tensor("bias", [K, F], F32, kind="ExternalInput")
    y = nc.dram_tensor("y", [F, B_LOC, L], F16, kind="ExternalOutput")

    groups = [list(range(N_CORES))]

    with tile.TileContext(nc) as tc:
        with (
            tc.tile_pool(name="const", bufs=1) as constp,
            tc.tile_pool(name="xres", bufs=N_GRP) as xres,
            tc.tile_pool(name="stats", bufs=1) as statsp,
            tc.tile_pool(name="pscr", bufs=2, space="PSUM") as pscr,
            tc.tile_pool(name="psmall", bufs=3, space="PSUM") as psmall,
            tc.tile_pool(name="dram", bufs=1, space="DRAM") as dram,
            tc.tile_pool(name="yout", bufs=4) as yout,
        ):
            # ---- warmup collective: absorbs the CC-stack first-op cost
            # (~64us) + launch skew while the x stream loads.
            warm_in = dram.tile([1, 8], F32)
            warm_out = dram.tile([K, 8], F32, addr_space="Shared")
            wz = statsp.tile([1, 8], F32)
            nc.gpsimd.memset(wz[:], 0.0)
            nc.gpsimd.dma_start(warm_in[:], wz[:])
            nc.gpsimd.collective_compute(
                "AllGather", ALU.bypass, replica_groups=groups,
                ins=[warm_in.opt()], outs=[warm_out.opt()])

            # const loads issue from the ACT sequencer so the x loads
            # lead the in-order Sync stream (PE cannot initiate DMAs).
            cpack1 = constp.tile([128, 128], F32)
            identt = cpack1[:, 0:128]
            nc.scalar.dma_start(identt, ident[:])
            cpack2 = constp.tile([B_LOC, 16], F32)
            mask64t = cpack2[:, 0:K]
            selTt = cpack2[:, K:2 * K]
            nc.scalar.dma_start(mask64t, mask64[:])
            nc.scalar.dma_start(selTt, selT[:])
            cpack4 = constp.tile([K, 321], F32)
            maskTt = cpack4[:, 0:B_LOC]
            wt = cpack4[:, 64:192]
            bt = cpack4[:, 192:320]
            epst = cpack4[:, 320:321]
            nc.scalar.dma_start(maskTt, maskT[:])
            nc.scalar.dma_start(wt, weight[:])
            nc.scalar.dma_start(bt, bias[:])
            nc.scalar.dma_start(epst, epsv[:])

            # ---- stats tiles (engine-major permuted column order) ----
            # cols 0:42  = DVE-lane batches (in group order)
            # cols 42:64 = ACT-lane batches (in group order)
            spackD = statsp.tile([128, 128 + NB_DVE], F32)
            Scol = spackD[:, 0:B_LOC]
            Qcol = spackD[:, B_LOC:128]
            CVcol = spackD[:, 128:128 + NB_DVE]
            # bn_stats raw out: [f, dve_batch, chunk, parity, (cnt,mean,cv)]
            BS = statsp.tile([128, NB_DVE, 2, 2, 3], F16)
            psq = statsp.tile([128, NB_DVE, 2, 2, 1], F32)
            # batch-major transposed stats: cols 0:128 = S^T, 128:256 = Q^T
            sqt = statsp.tile([B_LOC, 256], F32)
            gpart = statsp.tile([K, 256], F32)
            Gall = statsp.tile([N_CORES * K, 256], F32)
            gred = statsp.tile([K, 256], F32)
            postt = statsp.tile([K, 640], F32)
            selc = statsp.tile([128, 128], F32)

            # per-batch merge of bn_stats sub-chunk stats over a block of
            # DVE columns [lo, hi): sum(x)/256 = sum(means);
            # sum(x^2)/256 = sum(cv)/256 + sum(means^2).
            def merge(lo, hi):
                means = BS[:, lo:hi, :, :, 1:2]
                cvs = BS[:, lo:hi, :, :, 2:3]
                nc.vector.tensor_reduce(Scol[:, lo:hi], means, axis=AX.XYZ,
                                        op=ALU.add)
                nc.vector.tensor_mul(psq[:, lo:hi], means, means)
                nc.vector.tensor_reduce(Qcol[:, lo:hi], psq[:, lo:hi],
                                        axis=AX.XYZ, op=ALU.add)
                nc.vector.tensor_reduce(CVcol[:, lo:hi], cvs, axis=AX.XYZ,
                                        op=ALU.add)
                nc.vector.scalar_tensor_tensor(
                    Qcol[:, lo:hi], CVcol[:, lo:hi], 1.0 / 256.0,
                    Qcol[:, lo:hi], ALU.mult, ALU.add)

            # Half-group load DMAs (2 batches each) on two queues: the
            # engines start on half A of a group while half B is still in
            # flight, and the last-arriving half carries minimal work.
            res_tiles = {}
            for g in range(N_GRP):
                xtile = xres.tile([F, GRP * L], F16, tag="xs", name=f"xs{g}")
                res_tiles[g] = xtile
                for h in range(2):
                    ldq = nc.sync if h == 0 else nc.gpsimd
                    ldq.dma_start(
                        xtile[:, h * 2 * L:(h + 1) * 2 * L],
                        x[:, g * GRP + 2 * h:g * GRP + 2 * (h + 1), :])

            for g in range(N_GRP):
                xt = res_tiles[g]
                for i in range(N_DVE_G[g], GRP):
                    col = _stats_col(g, i)
                    xs = xt[:, i * L:(i + 1) * L]
                    # scale folds the 1/256 unit: (x/16)^2 and x/256.
                    scr = pscr.tile([128, L], F32, tag="ascr")
                    nc.scalar.activation(scr[:], xs, AFT.Square,
                                         scale=0.0625,
                                         accum_out=Qcol[:, col:col + 1])
                    scr2 = pscr.tile([128, L], F32, tag="ascr")
                    nc.scalar.activation(scr2[:], xs, AFT.Identity,
                                         scale=1.0 / 256.0,
                                         accum_out=Scol[:, col:col + 1])
                # bn_stats is capped at 512 free elems per op: 2 ops per
                # DVE batch
                for j in range(N_DVE_G[g]):
                    db = DVE_OFF[g] + j
                    i = j
                    for c in range(2):
                        nc.vector.bn_stats(
                            BS[:, db:db + 1, c:c + 1, :, :],
                            xt[:, (2 * i + c) * 512:(2 * i + c + 1) * 512])
                if g % MERGE_BLK == MERGE_BLK - 1:
                    blk = g // MERGE_BLK
                    merge(DVE_OFF[blk * MERGE_BLK],
                          DVE_OFF[(blk + 1) * MERGE_BLK])

            # Pre-load the Sqrt ACT table while ACT idles (the implicit
            # table swap would otherwise land on the post-AllGather
            # critical path).
            dumt = statsp.tile([K, 1], F32)
            nc.scalar.activation(dumt[:], epst, AFT.Sqrt)

            # ---- local per-class reduction: transpose + masked matmul ----
            sq_ps = psmall.tile([B_LOC, 256], F32, tag="ps")
            nc.tensor.transpose(sq_ps[:, 0:128], Scol, identt)
            nc.tensor.transpose(sq_ps[:, 128:256], Qcol, identt)
            nc.vector.tensor_copy(sqt[:], sq_ps[:])
            # gpart[k, 0:128] = partial mean, [k, 128:256] = partial E[x^2]
            # (mask64 carries the global 256/cnt factor, permuted rows).
            gp_ps = psmall.tile([K, 256], F32, tag="ps")
            nc.tensor.matmul(gp_ps[:], mask64t, sqt[:], start=True,
                             stop=True)
            nc.vector.tensor_copy(gpart[:], gp_ps[:])

            # ---- all-gather the [8, 256] partials across the 8 cores ----
            cc_in = dram.tile([K, 256], F32)
            cc_out = dram.tile([N_CORES * K, 256], F32, addr_space="Shared")
            # upload via GpSimd: a wait on the in-order Sync stream would
            # stall the stores queued there.
            nc.gpsimd.dma_start(cc_in[:], gpart[:])
            nc.gpsimd.collective_compute(
                "AllGather", ALU.bypass, replica_groups=groups,
                ins=[cc_in.opt()], outs=[cc_out.opt()])
            # Download issues from the ACT sequencer: it must wait for the
            # AllGather, and ACT is idle here anyway.
            nc.scalar.dma_start(Gall[:], cc_out[:])

            # ---- global stats + scale/shift per (class, feature) ----
            gr_ps = psmall.tile([K, 256], F32, tag="ps")
            nc.tensor.matmul(gr_ps[:], selTt, Gall[:], start=True,
                             stop=True)
            nc.vector.tensor_copy(gred[:], gr_ps[:])
            Gs = gred[:, 0:128]
            Gq = gred[:, 128:256]
            t0 = postt[:, 0:128]
            var = postt[:, 128:256]
            std = postt[:, 256:384]
            inv = postt[:, 384:512]
            scal = postt[:, 512:640]
            shft = t0  # reuse: t0 is dead once var is computed
            nc.vector.tensor_mul(t0, Gs, Gs)
            nc.vector.tensor_sub(var, Gq, t0)
            nc.scalar.activation(std, var, AFT.Sqrt, bias=epst)
            nc.vector.reciprocal(inv, std)
            nc.vector.tensor_mul(scal, inv, wt)

            # ---- select: [f, 0:64] = scale col, [f, 64:128] = shift col
            # (the scale-select matmul overlaps the shift math on DVE)
            sel_ps = psmall.tile([128, 2 * B_LOC], F32, tag="ps")
            nc.tensor.matmul(sel_ps[:, 0:B_LOC], scal, maskTt, start=True,
                             stop=True)
            nc.vector.tensor_mul(shft, Gs, scal)
            nc.vector.tensor_sub(shft, bt, shft)
            nc.tensor.matmul(sel_ps[:, B_LOC:2 * B_LOC], shft, maskTt,
                             start=True, stop=True)
            nc.vector.tensor_copy(selc[:], sel_ps[:])

            # ---- pass 2: y[:, b] = x[:, b] * ssel[:, b] + tsel[:, b] ----
            # All-DVE: ~0.5us/batch fp16 keeps compute well ahead of the
            # DMA drain; stores alternate between the gpsimd and sync
            # queues.
            for g in range(N_GRP):
                xt = res_tiles[g]
                yt = yout.tile([F, GRP * L], F16)
                for i in range(GRP):
                    b = g * GRP + i
                    nc.vector.tensor_scalar(yt[:, i * L:(i + 1) * L],
                                            xt[:, i * L:(i + 1) * L],
                                            selc[:, b:b + 1],
                                            selc[:, B_LOC + b:B_LOC + b + 1],
                                            ALU.mult, ALU.add)
                    if g == 0 and i == 0:
                        # first batch ships alone so the store stream
                        # ramps while the rest of the group computes
                        nc.gpsimd.dma_start(y[:, 0:1, :], yt[:, 0:L])
                if g == 0:
                    nc.sync.dma_start(y[:, 1:GRP, :], yt[:, L:GRP * L])
                    continue
                stq = (nc.gpsimd, nc.sync, nc.scalar)[g % 3]
                stq.dma_start(y[:, g * GRP:(g + 1) * GRP, :], yt[:])

    nc.finalize()
    return nc


def _get_nc():
    global _built
    if _built is None:
        _built = _build()
    return _built


def _host_inputs(x, labels, weight, bias):
    labels = np.asarray(labels).astype(np.int64)
    counts = np.bincount(labels, minlength=K).astype(np.float64) * L
    rcp = (256.0 / np.maximum(counts, 1.0)).astype(np.float32)  # [K]
    ident = np.eye(128, dtype=np.float32)
    selT = np.tile(np.eye(K, dtype=np.float32), (N_CORES, 1))  # [64, 8]
    # permutation: stats column p holds batch perm[p]
    perm = np.empty(B_LOC, dtype=np.int64)
    for g in range(N_GRP):
        for i in range(GRP):
            perm[_stats_col(g, i)] = g * GRP + i
    xh = np.asarray(x, dtype=np.float16)

    in_maps = []
    for c in range(N_CORES):
        lab = labels[c * B_LOC:(c + 1) * B_LOC]
        onehot = np.zeros((B_LOC, K), dtype=np.float32)
        onehot[np.arange(B_LOC), lab] = 1.0
        mask64 = onehot[perm] * rcp.reshape(1, K)
        in_maps.append({
            # feature-major shard: [F, B_LOC, L] fp16
            "x": np.ascontiguousarray(
                xh[c * B_LOC:(c + 1) * B_LOC].transpose(1, 0, 2)),
            "maskT": np.ascontiguousarray(onehot.T),
            "mask64": mask64,
            "selT": selT,
            "ident": ident,
            "epsv": np.full((K, 1), EPS, dtype=np.float32),
            "weight": np.ascontiguousarray(
                np.asarray(weight, dtype=np.float32)),
            "bias": np.ascontiguousarray(
                np.asarray(bias, dtype=np.float32)),
        })
    return in_maps


def run(x, labels, weight, bias, trace=False):
    nc = _get_nc()
    in_maps = _host_inputs(x, labels, weight, bias)
    res = bass_utils.run_bass_kernel_spmd(nc, in_maps, list(range(N_CORES)),
                                          trace=trace)
    out = np.concatenate(
        [res.results[c]["y"].transpose(1, 0, 2) for c in range(N_CORES)],
        axis=0).astype(np.float32)
    return out, res


def kernel(x, labels, weight, bias):
    out, _ = run(np.asarray(x, dtype=np.float32), labels,
                 np.asarray(weight, dtype=np.float32),
                 np.asarray(bias, dtype=np.float32))
    return out


# revision 28
# speedup vs baseline: 1.1290x; 1.1199x over previous
"""Conditional BatchNorm1d (training-mode, per-class stats) on 8 Trainium2
NeuronCores.

Problem: x [512, 128, 1024] f32, labels [512] i32 in [0,8), weight/bias
[8, 128] f32.  Per-class biased mean/var over the class's (batch, length)
elements per feature, then per-class affine:
    y = x * (rsqrt(var+eps)*w)[lbl] + (b - mean*rsqrt(var+eps)*w)[lbl]

Sharding: data-parallel over batch B across the 8 cores (64 batches each).

v6 (from v4 @154us): phase 1 was engine-bound (DVE ran 13us past the
last DMA).  (v5 tried an fp16 SBUF scratch to get the ACT lane to 2x
rate -- the perf mode does not engage for activation+accum, so the
22/42 ACT/DVE split stays.)
  * Merges run per 4-group block (4x5 wide DVE ops instead of 16x5
    tiny ones), cutting ~7us of DVE bubble overhead.
  * Stats columns live in engine-major permuted order (DVE batches in
    cols 0:32, ACT in 32:64) so block merges and the transposes stay
    contiguous; the host permutes the class-mask rows to match.
  * Consts issue from the idle PE sequencer (ACT sequencer fed the
    in-order const DMAs before).
  * Tiny warmup AllGather at t~1us eats the CC-stack first-op cost
    (~64us!) off the critical path; the real stats collective is an
    AllGather (12us flight vs 30us AllReduce) + local selection matmul.
  * Pass 2 all-DVE; loads/stores alternate sync/gpsimd DMA queues.

Layout: feature-major shard [F=128, B_LOC=64, L=1024] fp16; GRP=4
batches per DMA keeps 8 KiB of DRAM-contiguous data per partition.
"""

import sys

if "/opt/trn_rl_repo" not in sys.path:
    sys.path.insert(0, "/opt/trn_rl_repo")

import numpy as np

import concourse.bacc as bacc
import concourse.tile as tile
from concourse import mybir
from concourse import bass_utils

B, F, L = 512, 128, 1024
K = 8
N_CORES = 8
B_LOC = B // N_CORES  # 64
EPS = 1e-5
GRP = 4               # batches per DMA group (8 KiB/partition fp16)
N_GRP = B_LOC // GRP  # 16
# ACT covers the LAST n_act(g) batches of each group, DVE the first
# n_dve(g); 22 ACT / 42 DVE batches drains both engines with the DMA
# stream.  The 2-ACT-batch groups sit EARLY so the last-arriving half
# group carries the least tail work (DVE: 2 bn_stats batches, ACT: 1).
N_ACT_G = [2 if g < 6 else 1 for g in range(N_GRP)]
N_DVE_G = [GRP - n for n in N_ACT_G]
DVE_OFF = [0]
for g in range(N_GRP):
    DVE_OFF.append(DVE_OFF[-1] + N_DVE_G[g])
NB_DVE = DVE_OFF[-1]      # 42 DVE-lane batches
ACT_OFF = [NB_DVE]
for g in range(N_GRP):
    ACT_OFF.append(ACT_OFF[-1] + N_ACT_G[g])
MERGE_BLK = 4             # groups per merge block

F32 = mybir.dt.float32
F16 = mybir.dt.float16
AFT = mybir.ActivationFunctionType
AX = mybir.AxisListType
ALU = mybir.AluOpType

_built = None


def _stats_col(g, i):
    """Column in the permuted stats array for batch (group g, lane i)."""
    if i >= N_DVE_G[g]:
        return ACT_OFF[g] + (i - N_DVE_G[g])  # ACT region (last batches)
    return DVE_OFF[g] + i                     # DVE region (first batches)


def _build():
    nc = bacc.Bacc("TRN2", target_bir_lowering=False, debug=False,
                   num_devices=N_CORES)

    x = nc.dram_tensor("x", [F, B_LOC, L], F16, kind="ExternalInput")
    # One-hot label mask, transposed: maskT[k, b] = 1 iff labels[b] == k
    maskT = nc.dram_tensor("maskT", [K, B_LOC], F32, kind="ExternalInput")
    # Per-class stats mask in PERMUTED row order, rcp-scaled:
    # mask64[p, k] = 256/cnt_k iff labels[batch_at_col_p] == k.
    mask64 = nc.dram_tensor("mask64", [B_LOC, K], F32, kind="ExternalInput")
    # Cross-core sum-of-shards selector: selT[8r+k, k'] = (k == k').
    selT = nc.dram_tensor("selT", [N_CORES * K, K], F32,
                          kind="ExternalInput")
    ident = nc.dram_tensor("ident", [128, 128], F32, kind="ExternalInput")
    epsv = nc.dram_tensor("epsv", [K, 1], F32, kind="ExternalInput")
    weight = nc.dram_tensor("weight", [K, F], F32, kind="ExternalInput")
    bias = nc.dram_tensor("bias", [K, F], F32, kind="ExternalInput")
    y = nc.dram_tensor("y", [F, B_LOC, L], F16, kind="ExternalOutput")

    groups = [list(range(N_CORES))]

    with tile.TileContext(nc) as tc:
        with (
            tc.tile_pool(name="const", bufs=1) as constp,
            tc.tile_pool(name="xres", bufs=N_GRP) as xres,
            tc.tile_pool(name="stats", bufs=1) as statsp,
            tc.tile_pool(name="pscr", bufs=2, space="PSUM") as pscr,
            tc.tile_pool(name="psmall", bufs=3, space="PSUM") as psmall,
            tc.tile_pool(name="dram", bufs=1, space="DRAM") as dram,
            tc.tile_pool(name="yout", bufs=4) as yout,
        ):
            # ---- warmup collective: absorbs the CC-stack first-op cost
            # (~64us) + launch skew while the x stream loads.
            warm_in = dram.tile([1, 8], F32)
            warm_out = dram.tile([K, 8], F32, addr_space="Shared")
            wz = statsp.tile([1, 8], F32)
            nc.gpsimd.memset(wz[:], 0.0)
            nc.gpsimd.dma_start(warm_in[:], wz[:])
            nc.gpsimd.collective_compute(
                "AllGather", ALU.bypass, replica_groups=groups,
                ins=[warm_in.opt()], outs=[warm_out.opt()])

            # const loads issue from the ACT sequencer so the x loads
            # lead the in-order Sync stream (PE cannot initiate DMAs).
            cpack1 = constp.tile([128, 128], F32)
            identt = cpack1[:, 0:128]
            nc.scalar.dma_start(identt, ident[:])
            cpack2 = constp.tile([B_LOC, 16], F32)
            mask64t = cpack2[:, 0:K]
            selTt = cpack2[:, K:2 * K]
            nc.scalar.dma_start(mask64t, mask64[:])
            nc.scalar.dma_start(selTt, selT[:])
            cpack4 = constp.tile([K, 321], F32)
            maskTt = cpack4[:, 0:B_LOC]
            wt = cpack4[:, 64:192]
            bt = cpack4[:, 192:320]
            epst = cpack4[:, 320:321]
            nc.scalar.dma_start(maskTt, maskT[:])
            nc.scalar.dma_start(wt, weight[:])
            nc.scalar.dma_start(bt, bias[:])
            nc.scalar.dma_start(epst, epsv[:])

            # ---- stats tiles (engine-major permuted column order) ----
            # cols 0:42  = DVE-lane batches (in group order)
            # cols 42:64 = ACT-lane batches (in group order)
            spackD = statsp.tile([128, 128 + NB_DVE], F32)
            Scol = spackD[:, 0:B_LOC]
            Qcol = spackD[:, B_LOC:128]
            CVcol = spackD[:, 128:128 + NB_DVE]
            # bn_stats raw out: [f, dve_batch, chunk, parity, (cnt,mean,cv)]
            BS = statsp.tile([128, NB_DVE, 2, 2, 3], F16)
            psq = statsp.tile([128, NB_DVE, 2, 2, 1], F32)
            # batch-major transposed stats: cols 0:128 = S^T, 128:256 = Q^T
            sqt = statsp.tile([B_LOC, 256], F32)
            gpart = statsp.tile([K, 256], F32)
            Gall = statsp.tile([N_CORES * K, 256], F32)
            gred = statsp.tile([K, 256], F32)
            postt = statsp.tile([K, 640], F32)
            selc = statsp.tile([128, 128], F32)

            # per-batch merge of bn_stats sub-chunk stats over a block of
            # DVE columns [lo, hi): sum(x)/256 = sum(means);
            # sum(x^2)/256 = sum(cv)/256 + sum(means^2).
            def merge(lo, hi):
                means = BS[:, lo:hi, :, :, 1:2]
                cvs = BS[:, lo:hi, :, :, 2:3]
                nc.vector.tensor_reduce(Scol[:, lo:hi], means, axis=AX.XYZ,
                                        op=ALU.add)
                nc.vector.tensor_mul(psq[:, lo:hi], means, means)
                nc.vector.tensor_reduce(Qcol[:, lo:hi], psq[:, lo:hi],
                                        axis=AX.XYZ, op=ALU.add)
                nc.vector.tensor_reduce(CVcol[:, lo:hi], cvs, axis=AX.XYZ,
                                        op=ALU.add)
                nc.vector.scalar_tensor_tensor(
                    Qcol[:, lo:hi], CVcol[:, lo:hi], 1.0 / 256.0,
                    Qcol[:, lo:hi], ALU.mult, ALU.add)

            # Full-group load DMAs (8 KiB/partition descriptors -- smaller
            # descriptors measurably drop HBM efficiency) on two queues.
            res_tiles = {}
            for g in range(N_GRP):
                xtile = xres.tile([F, GRP * L], F16, tag="xs", name=f"xs{g}")
                res_tiles[g] = xtile
                ldq = nc.sync if g % 2 == 0 else nc.gpsimd
                ldq.dma_start(xtile[:], x[:, g * GRP:(g + 1) * GRP, :])

            for g in range(N_GRP):
                xt = res_tiles[g]
                for i in range(N_DVE_G[g], GRP):
                    col = _stats_col(g, i)
                    xs = xt[:, i * L:(i + 1) * L]
                    # scale folds the 1/256 unit: (x/16)^2 and x/256.
                    scr = pscr.tile([128, L], F32, tag="ascr")
                    nc.scalar.activation(scr[:], xs, AFT.Square,
                                         scale=0.0625,
                                         accum_out=Qcol[:, col:col + 1])
                    scr2 = pscr.tile([128, L], F32, tag="ascr")
                    nc.scalar.activation(scr2[:], xs, AFT.Identity,
                                         scale=1.0 / 256.0,
                                         accum_out=Scol[:, col:col + 1])
                # bn_stats is capped at 512 free elems per op: 2 ops per
                # DVE batch
                for j in range(N_DVE_G[g]):
                    db = DVE_OFF[g] + j
                    i = j
                    for c in range(2):
                        nc.vector.bn_stats(
                            BS[:, db:db + 1, c:c + 1, :, :],
                            xt[:, (2 * i + c) * 512:(2 * i + c + 1) * 512])
                if g % MERGE_BLK == MERGE_BLK - 1:
                    blk = g // MERGE_BLK
                    merge(DVE_OFF[blk * MERGE_BLK],
                          DVE_OFF[(blk + 1) * MERGE_BLK])

            # Pre-load the Sqrt ACT table while ACT idles (the implicit
            # table swap would otherwise land on the post-AllGather
            # critical path).
            dumt = statsp.tile([K, 1], F32)
            nc.scalar.activation(dumt[:], epst, AFT.Sqrt)

            # ---- local per-class reduction: transpose + masked matmul ----
            sq_ps = psmall.tile([B_LOC, 256], F32, tag="ps")
            nc.tensor.transpose(sq_ps[:, 0:128], Scol, identt)
            nc.tensor.transpose(sq_ps[:, 128:256], Qcol, identt)
            nc.vector.tensor_copy(sqt[:], sq_ps[:])
            # gpart[k, 0:128] = partial mean, [k, 128:256] = partial E[x^2]
            # (mask64 carries the global 256/cnt factor, permuted rows).
            gp_ps = psmall.tile([K, 256], F32, tag="ps")
            nc.tensor.matmul(gp_ps[:], mask64t, sqt[:], start=True,
                             stop=True)
            nc.vector.tensor_copy(gpart[:], gp_ps[:])

            # ---- all-gather the [8, 256] partials across the 8 cores ----
            cc_in = dram.tile([K, 256], F32)
            cc_out = dram.tile([N_CORES * K, 256], F32, addr_space="Shared")
            # upload via GpSimd: a wait on the in-order Sync stream would
            # stall the stores queued there.
            nc.gpsimd.dma_start(cc_in[:], gpart[:])
            nc.gpsimd.collective_compute(
                "AllGather", ALU.bypass, replica_groups=groups,
                ins=[cc_in.opt()], outs=[cc_out.opt()])
            # Download issues from the ACT sequencer: it must wait for the
            # AllGather, and ACT is idle here anyway.
            nc.scalar.dma_start(Gall[:], cc_out[:])

            # ---- global stats + scale/shift per (class, feature) ----
            gr_ps = psmall.tile([K, 256], F32, tag="ps")
            nc.tensor.matmul(gr_ps[:], selTt, Gall[:], start=True,
                             stop=True)
            nc.vector.tensor_copy(gred[:], gr_ps[:])
            Gs = gred[:, 0:128]
            Gq = gred[:, 128:256]
            t0 = postt[:, 0:128]
            var = postt[:, 128:256]
            std = postt[:, 256:384]
            inv = postt[:, 384:512]
            scal = postt[:, 512:640]
            shft = t0  # reuse: t0 is dead once var is computed
            nc.vector.tensor_mul(t0, Gs, Gs)
            nc.vector.tensor_sub(var, Gq, t0)
            nc.scalar.activation(std, var, AFT.Sqrt, bias=epst)
            nc.vector.reciprocal(inv, std)
            nc.vector.tensor_mul(scal, inv, wt)

            # ---- select: [f, 0:64] = scale col, [f, 64:128] = shift col
            # (the scale-select matmul overlaps the shift math on DVE)
            sel_ps = psmall.tile([128, 2 * B_LOC], F32, tag="ps")
            nc.tensor.matmul(sel_ps[:, 0:B_LOC], scal, maskTt, start=True,
                             stop=True)
            nc.vector.tensor_mul(shft, Gs, scal)
            nc.vector.tensor_sub(shft, bt, shft)
            nc.tensor.matmul(sel_ps[:, B_LOC:2 * B_LOC], shft, maskTt,
                             start=True, stop=True)
            nc.vector.tensor_copy(selc[:], sel_ps[:])

            # ---- pass 2: y[:, b] = x[:, b] * ssel[:, b] + tsel[:, b] ----
            # All-DVE: ~0.5us/batch fp16 keeps compute well ahead of the
            # DMA drain; stores alternate between the gpsimd and sync
            # queues.
            for g in range(N_GRP):
                xt = res_tiles[g]
                yt = yout.tile([F, GRP * L], F16)
                for i in range(GRP):
                    b = g * GRP + i
                    nc.vector.tensor_scalar(yt[:, i * L:(i + 1) * L],
                                            xt[:, i * L:(i + 1) * L],
                                            selc[:, b:b + 1],
                                            selc[:, B_LOC + b:B_LOC + b + 1],
                                            ALU.mult, ALU.add)
                    if g == 0 and i == 0:
                        # first batch ships alone so the store stream
                        # ramps while the rest of the group computes
                        nc.gpsimd.dma_start(y[:, 0:1, :], yt[:, 0:L])
                if g == 0:
                    nc.sync.dma_start(y[:, 1:GRP, :], yt[:, L:GRP * L])
                    continue
                stq = (nc.gpsimd, nc.sync, nc.scalar)[g % 3]
                stq.dma_start(y[:, g * GRP:(g + 1) * GRP, :], yt[:])

    nc.finalize()
    return nc


def _get_nc():
    global _built
    if _built is None:
        _built = _build()
    return _built


def _host_inputs(x, labels, weight, bias):
    labels = np.asarray(labels).astype(np.int64)
    counts = np.bincount(labels, minlength=K).astype(np.float64) * L
    rcp = (256.0 / np.maximum(counts, 1.0)).astype(np.float32)  # [K]
    ident = np.eye(128, dtype=np.float32)
    selT = np.tile(np.eye(K, dtype=np.float32), (N_CORES, 1))  # [64, 8]
    # permutation: stats column p holds batch perm[p]
    perm = np.empty(B_LOC, dtype=np.int64)
    for g in range(N_GRP):
        for i in range(GRP):
            perm[_stats_col(g, i)] = g * GRP + i
    xh = np.asarray(x, dtype=np.float16)

    in_maps = []
    for c in range(N_CORES):
        lab = labels[c * B_LOC:(c + 1) * B_LOC]
        onehot = np.zeros((B_LOC, K), dtype=np.float32)
        onehot[np.arange(B_LOC), lab] = 1.0
        mask64 = onehot[perm] * rcp.reshape(1, K)
        in_maps.append({
            # feature-major shard: [F, B_LOC, L] fp16
            "x": np.ascontiguousarray(
                xh[c * B_LOC:(c + 1) * B_LOC].transpose(1, 0, 2)),
            "maskT": np.ascontiguousarray(onehot.T),
            "mask64": mask64,
            "selT": selT,
            "ident": ident,
            "epsv": np.full((K, 1), EPS, dtype=np.float32),
            "weight": np.ascontiguousarray(
                np.asarray(weight, dtype=np.float32)),
            "bias": np.ascontiguousarray(
                np.asarray(bias, dtype=np.float32)),
        })
    return in_maps


def run(x, labels, weight, bias, trace=False):
    nc = _get_nc()
    in_maps = _host_inputs(x, labels, weight, bias)
    res = bass_utils.run_bass_kernel_spmd(nc, in_maps, list(range(N_CORES)),
                                          trace=trace)
    out = np.concatenate(
        [res.results[c]["y"].transpose(1, 0, 2) for c in range(N_CORES)],
        axis=0).astype(np.float32)
    return out, res


def kernel(x, labels, weight, bias):
    out, _ = run(np.asarray(x, dtype=np.float32), labels,
                 np.asarray(weight, dtype=np.float32),
                 np.asarray(bias, dtype=np.float32))
    return out
